# revision 23
# baseline (speedup 1.0000x reference)
"""Trainium2 Bass kernel for the CubeSimulator problem (v2).

Reference: rotate (96,96,96) grids, per-voxel line-of-sight velocity u and
intensity I, Gaussian-KDE cube[i,j,v] = norm * sum_z exp(-(vel_v-u)^2/sig^2)*I,
then trilinear downsample (96,96,64) -> (32,64,64).

Exact structural simplifications (from the v1 baseline, validated):
 - axis0 96->32 downsample is a pure row selection (rows 3k+1);
 - axis2 64->64 downsample is the identity;
 - axis1 96->64 is an exact 2-tap stencil 0.75/0.25 (even) 0.25/0.75 (odd).

New in v2 (all validated numerically against the reference, ~1.3e-3 rel):
 - Gaussian-frame factorization: K_sig(vv-u) ~= sum_k M[k,v] * K_sig'(w_k-u)
   with sig'^2 = sig^2/2, on a coarse w-grid (NK ~= 24 << 64 bins).  M is a
   tiny host-side least-squares fit -- it only depends on runtime scalars
   (sigma, vel grid), like the baseline's bc/wj constants.
 - Multiplicative recurrence: within a chain of bins, E~_{k+1} = E~_k * F
   with F = exp(dw*2u/sig'^2), so each bin costs ONE VectorE/GpSimd multiply
   instead of an arg-build + ScalarE exp.  Chains are re-anchored with a
   fresh exp every few bins so fp32 underflow at chain starts cannot
   corrupt later bins (error bound ~e^{-87+span} << tolerance).  The
   per-bin scalar exp(-(w_k^2-w_anchor^2)/sig'^2) folds into M's rows.
 - z-reduction fused with the frame matrix: per bin one TensorE matmul
   with stationary [z=96, v=64] = M~[k,:] replicated (fp32r: 1 cycle/row
   since the moving fp32r operand is 384 wide), accumulating the final
   [v=64, (i,j)=384] cube directly in PSUM.  No per-i-row matmuls, no
   second downsample matmul.
 - j-downsample as two strided scalar_tensor_tensor stencil ops reading
   PSUM directly (3*a+b form, the 0.25 folds into M).
 - column-split pipelining (halves) so the second half's prep overlaps the
   first half's KDE chains.

Fallback: for degenerate runtime scalars (tiny sigma, wild velocity grids)
the same machinery runs in "direct" mode: w = velocity grid, sig' = sig,
M = norm*I, chains of length 1 (anchor-only) -- mathematically exact.

Sharding: 32 needed i-rows split 4-per-core across 8 cores; per-core device
layout [z=96 partitions, (i=4)x(j=96)=384 free].
"""

import math

import numpy as np

import concourse.bacc as bacc
import concourse.bass as bass
import concourse.mybir as mybir
import concourse.tile as tile
from concourse.bass_utils import run_bass_kernel_spmd

G = 96
NV = 64
N_CORES = 8
OUT_I = 32
ROWS_PER_CORE = OUT_I // N_CORES   # 4
PX = ROWS_PER_CORE * G             # 384
OUT_J = 64

F32 = mybir.dt.float32
F32R = mybir.dt.float32r
AF = mybir.ActivationFunctionType
OP = mybir.AluOpType

LAST_EXEC_NS = None

# tuning knobs
NSPLIT = 2            # column pipelining splits of the 384 free dim
CHAIN = 6             # bins per anchor chain
VG_PATTERN = None     # optional explicit engine pattern for KDE ops


def _plan_frame(si, sig2, vel):
    """Host-side: choose mode, w-grid, chain anchors and the M~ matrix.

    Returns (w, anchors, M) where M is (NK, NV) float32 with all scalar
    folds applied (norm, 0.25 stencil fold, per-chain recentering), or the
    direct-mode equivalent (w = vel, anchors = every bin, M = diag-ish).
    """
    sig = math.sqrt(sig2)
    norm = 1.0 / math.sqrt(2.0 * math.pi * sig2)
    umax = 200.0 * abs(si)
    sp2 = sig2 / 2.0
    sp = sig / math.sqrt(2.0)
    dw = 0.9 * sp
    span_lo = -umax - 1.0 * sp
    span_hi = umax + 1.0 * sp
    nk = int(math.ceil((span_hi - span_lo) / dw)) + 1
    nk = max(nk, 4)

    mode = "frame"
    if nk > 40:
        mode = "direct"
    else:
        w = np.linspace(span_lo, span_hi, nk)
        ddw = float(w[1] - w[0]) if nk > 1 else 0.0
        # F = exp(ddw*2u/sp2) must stay finite, and the per-chain recentered
        # running product bounded by e^{(w_k^2-w_a^2)/sp2} < e^75
        if ddw * 2.0 * umax / sp2 > 60.0:
            mode = "direct"
        wsq = w * w
        for a in range(0, nk, CHAIN):
            last = min(a + CHAIN, nk) - 1
            if abs(wsq[last] - wsq[a]) / sp2 > 75.0:
                mode = "direct"
                break

    velf = vel.astype(np.float64)
    if mode == "direct":
        w = velf.copy()
        anchors = list(range(len(w)))
        M = np.eye(len(w), NV) * norm
        return mode, w, anchors, M.astype(np.float64), sig2

    # least-squares fit of the 64 target kernels in the frame
    uu = max(umax, 2.0 * sp)
    us = np.linspace(-uu, uu, 4001)
    Phi = np.exp(-((us[:, None] - w[None, :]) ** 2) / sp2)
    T = np.exp(-((velf[None, :] - us[:, None]) ** 2) / sig2)
    M = np.linalg.solve(Phi.T @ Phi + 1e-8 * np.eye(nk), Phi.T @ T)
    M *= norm
    anchors = list(range(0, nk, CHAIN))
    # recentering fold: row k of chain anchored at a: M~ = M*exp(-(w_k^2-w_a^2)/sp2)
    for a in anchors:
        for k in range(a, min(a + CHAIN, nk)):
            M[k, :] *= math.exp(-(w[k] ** 2 - w[a] ** 2) / sp2)
    return mode, w, anchors, M, sp2


def _build_program(ci, si, cr, sr, sig2, vel):
    vel = np.asarray(vel, np.float32).reshape(-1)
    mode, w, anchors, M, sp2 = _plan_frame(si, sig2, vel)
    nk = len(w)
    anchor_of = {}
    for a in anchors:
        for k in range(a, min(a + CHAIN, nk) if mode == "frame" else a + 1):
            anchor_of[k] = a

    nc = bacc.Bacc("TRN2")

    # x,y,z packed as [G, (nsplit, 3, PX/nsplit)] so each half is one DMA
    gs = nc.dram_tensor("gs", [G, 3 * PX], F32, kind="ExternalInput")
    # M~ replicated down the z-partitions: [96, nk*64]
    ms = nc.dram_tensor("ms", [G, nk * NV], F32R, kind="ExternalInput")
    # per-anchor exp biases -w_a^2/sp2 (+ trailing Ln clamp), replicated
    na = len(anchors)
    bc = nc.dram_tensor("bc", [G, na + 1], F32, kind="ExternalInput")
    out = nc.dram_tensor("out", [NV, PX // 3 * 2], F32, kind="ExternalOutput")

    usc = -200.0 * si              # u = usc * cr-folded * tanh(r/2)*rx'/r
    spv = float(sp2)

    with tile.TileContext(nc) as tc:
        with (
            tc.tile_pool(name="io", bufs=1) as io,
            tc.tile_pool(name="prep", bufs=1) as prep,
            tc.tile_pool(name="kde", bufs=2) as kde,
            tc.tile_pool(name="psum", bufs=1, space="PSUM") as psum,
        ):
            hw0 = PX // NSPLIT
            g3 = io.tile([G, 3, NSPLIT, hw0], F32, tag="g3")
            for h in range(NSPLIT):
                nc.sync.dma_start(
                    out=g3[:, :, h, :],
                    in_=gs[:, h * 3 * hw0:(h + 1) * 3 * hw0])
            mst = io.tile([G, nk * NV], F32R, tag="mst")
            nc.gpsimd.dma_start(out=mst[:], in_=ms[:])
            bct = io.tile([G, na + 1], F32, tag="bct")
            nc.scalar.dma_start(out=bct[:], in_=bc[:])
            aidx = {a: i for i, a in enumerate(anchors)}
            # (B) one explicit act-table load covering ln/exp/abs/square
            nc.scalar.add_instruction(mybir.InstLoadActFuncSet(
                name=nc.get_next_instruction_name(), act_func_set_id=6,
                ins=[], outs=[]))

            def vtile(name):
                return prep.tile([G, PX], F32, tag=name, name=name)

            t0 = vtile("t0")
            rxp = vtile("rxp")
            ryp = vtile("ryp")
            rzp = vtile("rzp")
            sqx = vtile("sqx")
            sqy = vtile("sqy")
            q = vtile("q")
            lnq = vtile("lnq")
            r = vtile("r")
            er = vtile("er")
            ed = vtile("ed")
            rec = vtile("rec")
            t1 = vtile("t1")
            u0f = vtile("u0f")
            u0 = vtile("u0")
            az = vtile("az")
            h1 = vtile("h1")
            ssq = vtile("ssq")
            A2 = vtile("A2")
            Ft = vtile("Ft") if mode == "frame" else None

            cube = psum.tile([NV, PX], F32)
            out_sb = io.tile([NV, PX // 3 * 2], F32, tag="out_sb")

            nsp = NSPLIT
            cols = [(h * PX // nsp, (h + 1) * PX // nsp) for h in range(nsp)]

            # ddw for F; in direct mode chains have length 1 and F is unused
            ddw = float(w[1] - w[0]) if (mode == "frame" and nk > 1) else 0.0

            mm_emitted = 0

            def emit_matmul(k, et):
                nonlocal mm_emitted
                nc.tensor.matmul(
                    cube[:],
                    mst[:, k * NV:(k + 1) * NV],
                    et[:],
                    start=(mm_emitted == 0), stop=(mm_emitted == nk - 1),
                    skip_group_check=True,
                )
                mm_emitted += 1

            # u = uk*u0 with uk = usc*cr ; w-term scalar on u0 is
            # w*2*uk/sp2 ; F = exp(ddw*2*uk/sp2 * u0)
            uk = usc * cr
            V, Gp, S = nc.vector, nc.gpsimd, nc.scalar
            maxlen = CHAIN if mode == "frame" else 1
            ets = {}
            echain = {}
            vgi = 0
            mm_left = set(range(nk))
            for h, (c0, c1) in enumerate(cols):
                sl = slice(c0, c1)
                xt = g3[:, 0, h, :]
                yt = g3[:, 1, h, :]
                zt = g3[:, 2, h, :]
                V.scalar_tensor_tensor(
                    t0[:, sl], xt, sr / cr, yt, OP.mult, OP.add)
                V.scalar_tensor_tensor(
                    rxp[:, sl], yt, -sr / cr, xt, OP.mult, OP.add)
                V.scalar_tensor_tensor(
                    ryp[:, sl], zt, -si / (ci * cr), t0[:, sl],
                    OP.mult, OP.add)
                V.scalar_tensor_tensor(
                    rzp[:, sl], zt, ci / (si * cr) if si != 0 else 0.0,
                    t0[:, sl], OP.mult, OP.add)
                V.scalar_tensor_tensor(
                    sqx[:, sl], rxp[:, sl], cr * cr, rxp[:, sl],
                    OP.mult, OP.mult)
                Gp.tensor_mul(sqy[:, sl], ryp[:, sl], ryp[:, sl])
                V.scalar_tensor_tensor(
                    q[:, sl], sqy[:, sl], (ci * cr) ** 2, sqx[:, sl],
                    OP.mult, OP.add)
                S.activation(lnq[:, sl], q[:, sl], AF.Ln,
                             bias=bct[0:G, na:na + 1])
                S.activation(r[:, sl], lnq[:, sl], AF.Exp, scale=0.5)
                S.activation(er[:, sl], r[:, sl], AF.Exp)
                # den = (er+1)*r ; u0 = rxp*(er-1)/den ; u = uk*u0
                V.scalar_tensor_tensor(
                    ed[:, sl], er[:, sl], 1.0, r[:, sl], OP.add, OP.mult)
                V.reciprocal(rec[:, sl], ed[:, sl])
                Gp.tensor_mul(t1[:, sl], er[:, sl], rxp[:, sl])
                V.scalar_tensor_tensor(
                    u0f[:, sl], rxp[:, sl], -1.0, t1[:, sl], OP.mult, OP.add)
                V.scalar_tensor_tensor(
                    u0[:, sl], u0f[:, sl], 1.0, rec[:, sl], OP.mult, OP.mult)
                # A2 = -(h1)/3 - ssq ; h1 = 6|si*cr||rzp| + r
                S.activation(az[:, sl], rzp[:, sl], AF.Abs,
                             scale=6.0 * abs(si * cr))
                Gp.tensor_add(h1[:, sl], az[:, sl], r[:, sl])
                V.scalar_tensor_tensor(
                    ssq[:, sl], u0[:, sl], uk * uk / spv, u0[:, sl],
                    OP.mult, OP.mult)
                V.scalar_tensor_tensor(
                    A2[:, sl], h1[:, sl], -1.0 / 3.0, ssq[:, sl],
                    OP.mult, OP.subtract)
                if mode == "frame":
                    S.activation(Ft[:, sl], u0[:, sl], AF.Exp,
                                 scale=ddw * 2.0 * uk / spv)

                # --- KDE for this half: anchors + chains, wave order ---
                for step in range(maxlen):
                    for a in anchors:
                        k = a + step
                        if k >= nk or anchor_of.get(k) != a:
                            continue
                        if h == 0:
                            ets[k] = kde.tile([G, PX], F32R,
                                              tag="e%d" % k, bufs=1,
                                              name="e%d" % k)
                        et = ets[k]
                        if step == 0:
                            arg = kde.tile([G, PX], F32, tag="arg", bufs=4,
                                           name="arg")
                            nc.vector.scalar_tensor_tensor(
                                arg[:, sl], u0[:, sl],
                                float(w[a]) * 2.0 * uk / spv,
                                A2[:, sl], OP.mult, OP.add)
                            nc.scalar.activation(
                                et[:, sl], arg[:, sl], AF.Exp,
                                bias=bct[0:G, aidx[a]:aidx[a] + 1])
                        else:
                            ep = echain[(a, h)]
                            eng = nc.gpsimd if (vgi % 2 == 1) else nc.vector
                            vgi += 1
                            eng.tensor_mul(et[:, sl], ep[:, sl], Ft[:, sl])
                        echain[(a, h)] = et
                        if h == NSPLIT - 1:
                            emit_matmul(k, et)
                            mm_left.discard(k)

            # --- j-downsample stencil straight out of PSUM ---
            # even jj=2m: 0.75*c[3m] + 0.25*c[3m+1] = (3*c[3m]+c[3m+1])/4
            # odd  jj=2m+1: (3*c[3m+2]+c[3m+1])/4 ; the /4 is folded into M
            cube_sb = io.tile([NV, PX], F32, tag="cube_sb")
            nc.vector.tensor_copy(cube_sb[:], cube[:])
            nc.vector.scalar_tensor_tensor(
                out_sb[:, 0:PX // 3 * 2:2], cube_sb[:, 0:PX:3], 3.0,
                cube_sb[:, 1:PX:3], OP.mult, OP.add)
            nc.vector.scalar_tensor_tensor(
                out_sb[:, 1:PX // 3 * 2:2], cube_sb[:, 2:PX:3], 3.0,
                cube_sb[:, 1:PX:3], OP.mult, OP.add)
            nc.sync.dma_start(out=out[:], in_=out_sb[:])

    return nc, mode, w, anchors, M, sp2


def _host_constants(M, nk):
    # M~ scaled by stencil fold 0.25, replicated down 96 partitions
    Mr = (0.25 * M).astype(np.float32)            # (nk, 64)
    return np.ascontiguousarray(
        np.tile(Mr.reshape(1, nk * NV), (G, 1)))


def kernel(**inputs):
    inc = float(np.asarray(inputs["inclination"]).reshape(-1)[0])
    rot = float(np.asarray(inputs["sky_rot"]).reshape(-1)[0])
    lb = float(np.asarray(inputs["line_broadening"]).reshape(-1)[0])
    vel = np.asarray(inputs["velocity_grid"], np.float32).reshape(-1)
    X = np.asarray(inputs["Xgrid"], np.float32)
    Y = np.asarray(inputs["Ygrid"], np.float32)
    Z = np.asarray(inputs["Zgrid"], np.float32)

    ci, si = math.cos(inc), math.sin(inc)
    cr, sr = math.cos(rot), math.sin(rot)
    sig2 = float(np.float32(lb) * np.float32(lb))
    if not (sig2 > 0.0) or not math.isfinite(sig2):
        sig2 = 1e-30

    nc, mode, w, anchors, M, sp2 = _build_program(ci, si, cr, sr, sig2, vel)
    nc.finalize()

    msv = _host_constants(M, len(w))
    bcv = np.ascontiguousarray(np.tile(
        np.asarray([-w[a] * w[a] / sp2 for a in anchors] + [1e-30],
                   np.float32).reshape(1, -1), (G, 1)))

    in_maps = []
    for c in range(N_CORES):
        rows = [3 * k + 1 for k in range(ROWS_PER_CORE * c,
                                         ROWS_PER_CORE * (c + 1))]
        def shard(a):
            s = a[rows]                              # (4, 96, 96) = (i, j, z)
            s = s.transpose(2, 0, 1).reshape(G, PX)  # [z, i*96+j]
            return s
        hw0 = PX // NSPLIT
        # [z, (nsplit, 3, hw0)] so each half of x/y/z is one contiguous DMA
        g = np.stack([shard(X), shard(Y), shard(Z)], axis=1)  # [z, 3, PX]
        g = g.reshape(G, 3, NSPLIT, hw0).transpose(0, 2, 1, 3)
        gsv = np.ascontiguousarray(g.reshape(G, 3 * PX))
        in_maps.append({"gs": gsv, "ms": msv, "bc": bcv})

    res = run_bass_kernel_spmd(nc, in_maps, core_ids=list(range(N_CORES)))
    global LAST_EXEC_NS
    LAST_EXEC_NS = res.exec_time_ns

    parts = []
    for c in range(N_CORES):
        o = res.results[c]["out"]                    # (64, 256) = [v,(i,m,p)]
        parts.append(o.reshape(NV, ROWS_PER_CORE, OUT_J)
                      .transpose(1, 2, 0))           # (4, 64jj, 64v)
    return np.concatenate(parts, axis=0).astype(np.float32)  # (32, 64, 64)
